# revision 44
# baseline (speedup 1.0000x reference)
"""Causal attention (B=2, L=2048, H=8, E=64) returning (V, SA) on 8 trn2 NeuronCores.

Sharding: the 16 (b,h) pairs are split 2-per-core (batch*head data parallel);
each core holds full L so the causal softmax needs no communication. Only the
lower causal triangle is ever computed or written (the strictly-upper triangle
of SA is never touched: PJRT donates zero-initialized output buffers, so those
regions read back as exact 0).

Per-core pipeline (matmuls bf16 on PE, f32 PSUM accumulation), fully fused over
steps i = 0..15 (row-tile i = 128 query rows), both heads interleaved:
  phase1: S row-tile = Q^T_i.T @ K^T chunks -> additive -1e9 mask on the
          diagonal block (DVE, pre-exp so row sums come out right) -> one ACT
          exp(0.125*x) per PSUM part with accum_out row-sums -> reciprocal ->
          normalize in place (DVE) -> DMA SA rows out immediately.
  phase2: E^T produced directly by recomputing the scores transposed
          (lhsT/rhs swapped) + exp — cheaper than any on-chip transpose path
          (PE-identity transpose needs a PSUM drain pass; DMA transpose is
          packet-rate-bound). Emitted in 1024-wide parts on the causal
          diagonal schedule (part p of col-tile j at step j+8p, just before
          first use) to keep per-step PE/ACT work flat.
  phase3: O row-tile = sum_j E^T-block @ V_j, lagged one step behind phase2
          so the in-order PE never stalls on fresh exp output; normalize by
          the phase-1 reciprocals, DMA out.
Within each step, phase2 is emitted BEFORE phase1 (and phase3 last): measured
~5us faster — prioritizing E^T production keeps the next step's O-matmul
inputs ahead of the pipeline. Input loads are one DMA per tensor with issue
work split across both HWDGE queues (Q/K on the scalar queue, V on sync):
twelve split issues serialized ~10us on one queue and cost ~2us of wall time.
"""
import sys
import types

sys.path.insert(0, "/opt/trn_rl_repo")

import numpy as np
import ml_dtypes

B, L, H, E = 2, 2048, 8, 64
P = 128          # partitions / l-tile size
T = L // P       # 16 tiles
NBH = 2          # (b,h) pairs per core
N_CORES = 8
SCALE = 1.0 / 8.0
NEG = -1.0e9

_CACHE = {}


def _install_ntff_hook():
    """The image's antenv lacks axon_hooks; shim it so trace=True works."""
    import antenv

    if "antenv.axon_hooks" in sys.modules:
        return
    hooks = types.ModuleType("antenv.axon_hooks")
    hooks._hook = None
    hooks.set_axon_ntff_profile_hook = lambda h: setattr(hooks, "_hook", h)
    hooks.get_axon_ntff_profile_hook = lambda: hooks._hook
    sys.modules["antenv.axon_hooks"] = hooks
    antenv.axon_hooks = hooks
    try:
        from trn_agent_boot.trn_boot import _ntff_profile_via_ctypes

        hook = _ntff_profile_via_ctypes("/opt/axon/libaxon_pjrt.so")
        if hook is not None:
            hooks.set_axon_ntff_profile_hook(hook)
    except Exception:
        pass


def _emit(nc, tc, ctx, aps, mybir):
    import concourse.tile as tile  # noqa: F401

    f32 = mybir.dt.float32
    bf16 = mybir.dt.bfloat16
    Exp = mybir.ActivationFunctionType.Exp

    q = aps["q"].rearrange("n (t p) e -> n p t e", p=P)
    k = aps["k"].rearrange("n (t p) e -> n p t e", p=P)
    v = aps["v"].rearrange("n (t p) e -> n p t e", p=P)
    sa = aps["sa"].rearrange("n (t p) s -> n t p s", p=P)
    o = aps["o"].rearrange("n (t p) e -> n t p e", p=P)

    # packed-causal column offsets inside the E / E^T sbuf tiles
    offE = [P * (i * (i + 1) // 2) for i in range(T + 1)]            # row-tile i: width (i+1)*P
    offT = [P * (T * j - j * (j - 1) // 2) for j in range(T + 1)]    # col-tile j: width (T-j)*P
    CW = offE[T]  # 17408

    cpool = ctx.enter_context(tc.tile_pool(name="consts", bufs=1))
    ldpool = ctx.enter_context(tc.tile_pool(name="ld", bufs=6))
    bfpool = ctx.enter_context(tc.tile_pool(name="bfp", bufs=2))
    qktpool = ctx.enter_context(tc.tile_pool(name="qkt", bufs=2))
    bigpool = ctx.enter_context(tc.tile_pool(name="big", bufs=2))
    outpool = ctx.enter_context(tc.tile_pool(name="outp", bufs=8))
    smallpool = ctx.enter_context(tc.tile_pool(name="small", bufs=4))
    pspool = ctx.enter_context(tc.tile_pool(name="ps", bufs=3, space="PSUM"))
    popool = ctx.enter_context(tc.tile_pool(name="po", bufs=2, space="PSUM"))

    PSW = 1024  # S/ST psum tile width

    mask_sb = cpool.tile([P, P], f32, tag="mask")
    nc.sync.dma_start(mask_sb[:], aps["maskneg"][:])
    maskT_sb = cpool.tile([P, P], f32, tag="maskT")
    nc.sync.dma_start(maskT_sb[:], aps["masknegT"][:])
    ident_sb = cpool.tile([P, P], bf16, tag="ident")
    nc.sync.dma_start(ident_sb[:], aps["ident"][:])

    qt = {}
    kt = {}
    vb = {}
    et = {}
    recips = {}
    for bh in range(NBH):
        # ---- load + cast ----
        qf = ldpool.tile([P, T, E], f32, tag="ld")
        nc.scalar.dma_start(qf[:], q[bh])
        kf = ldpool.tile([P, T, E], f32, tag="ld")
        nc.scalar.dma_start(kf[:], k[bh])
        vf = ldpool.tile([P, T, E], f32, tag="ld")
        nc.sync.dma_start(vf[:], v[bh])

        qb = bfpool.tile([P, T, E], bf16, tag="qb")
        nc.vector.tensor_copy(qb[:], qf[:])
        kb = bfpool.tile([P, T, E], bf16, tag="kb")
        nc.vector.tensor_copy(kb[:], kf[:])
        vb[bh] = bfpool.tile([P, T, E], bf16, tag="vb", name=f"vb{bh}")
        nc.vector.tensor_copy(vb[bh][:], vf[:])

        # ---- Q^T / K^T via matmul-by-identity (one batched PSUM->SBUF cast) ----
        qt[bh] = qktpool.tile([E, L], bf16, tag="qt", name=f"qt{bh}")
        kt[bh] = qktpool.tile([E, L], bf16, tag="kt", name=f"kt{bh}")
        for src, dst in ((qb, qt[bh]), (kb, kt[bh])):
            pt = pspool.tile([E, PSW], f32, tag="ps")
            for t in range(8):
                nc.tensor.matmul(
                    pt[:, t * P : (t + 1) * P], src[:, t, :], ident_sb[:],
                    start=True, stop=True,
                )
            nc.vector.tensor_copy(dst[:, 0 : 8 * P], pt[:, 0 : 8 * P])
            pt2 = pspool.tile([E, PSW], f32, tag="ps")
            for t in range(8, T):
                tt = t - 8
                nc.tensor.matmul(
                    pt2[:, tt * P : (tt + 1) * P],
                    src[:, t, :], ident_sb[:], start=True, stop=True,
                )
            nc.vector.tensor_copy(dst[:, 8 * P : L], pt2[:, 0 : L - 8 * P])

        et[bh] = bigpool.tile([P, CW], bf16, tag="et", name=f"et{bh}")
        recips[bh] = smallpool.tile([P, T], f32, tag="recips", name=f"recips{bh}")

    # ---- fused main loop. Step i emits:
    #  * phase1(i): S row-tile i -> mask -> exp(+rowsum accum) -> normalize -> DMA SA
    #  * phase2 parts on the diagonal: 1024-wide row-parts (j, p) with j+8p == i,
    #    exactly when first needed, so per-step ACT/PE work stays flat instead of
    #    front-loading the wide S^T column-tiles
    #  * phase3(i-1): O row-tile lagged one step so its E^T inputs are a step old
    #    and the in-order PE never stalls waiting on fresh exp output
    def phase3(ip):
        for bh in range(NBH):
            po = popool.tile([P, E], f32, tag="po")
            for j3 in range(ip + 1):
                nc.tensor.matmul(
                    po[:],
                    et[bh][:, offT[j3] + (ip - j3) * P : offT[j3] + (ip - j3 + 1) * P],
                    vb[bh][:, j3, :],
                    start=(j3 == 0),
                    stop=(j3 == ip),
                )
            o_st = outpool.tile([P, E], f32, tag="o")
            nc.vector.tensor_scalar_mul(
                o_st[:], po[:], recips[bh][:, ip : ip + 1]
            )
            nc.sync.dma_start(o[bh, ip], o_st[:])

    for i in range(T):
        W = (i + 1) * P
        for bh in range(NBH):
            # phase 2 first
            # phase 2 parts: (j, p) with j + 8p == i
            for (j, pp) in ([(i, 0)] + ([(i - 8, 1)] if i >= 8 else [])):
                Wj = (T - j) * P
                p0 = pp * PSW
                if p0 >= Wj:
                    continue
                pw = min(PSW, Wj - p0)
                base = j * P
                ps = pspool.tile([P, PSW], f32, tag="ps")
                for c0 in range(0, pw, 512):
                    w = min(512, pw - c0)
                    nc.tensor.matmul(
                        ps[:, c0 : c0 + w],
                        kt[bh][:, base : base + P],
                        qt[bh][:, base + p0 + c0 : base + p0 + c0 + w],
                        start=True,
                        stop=True,
                    )
                if pp == 0:  # diag block is the first 128 cols of part 0
                    nc.vector.tensor_add(ps[:, 0:P], ps[:, 0:P], maskT_sb[:])
                nc.scalar.activation(
                    et[bh][:, offT[j] + p0 : offT[j] + p0 + pw],
                    ps[:, 0:pw], Exp, scale=SCALE,
                )

            # phase 1
            sums = smallpool.tile([P, 2], f32, tag="sums")
            sa_st = outpool.tile([P, 2048], f32, tag="sa")
            nparts = 0
            for p0 in range(0, W, PSW):
                pw = min(PSW, W - p0)
                ps = pspool.tile([P, PSW], f32, tag="ps")
                for c0 in range(0, pw, 512):
                    w = min(512, pw - c0)
                    nc.tensor.matmul(
                        ps[:, c0 : c0 + w],
                        qt[bh][:, i * P : (i + 1) * P],
                        kt[bh][:, p0 + c0 : p0 + c0 + w],
                        start=True,
                        stop=True,
                    )
                if p0 <= i * P < p0 + pw:  # diag block lives in this psum tile
                    d0 = i * P - p0
                    nc.vector.tensor_add(
                        ps[:, d0 : d0 + P], ps[:, d0 : d0 + P], mask_sb[:]
                    )
                nc.scalar.activation(
                    sa_st[:, p0 : p0 + pw], ps[:, 0:pw], Exp, scale=SCALE,
                    accum_out=sums[:, nparts : nparts + 1],
                )
                nparts += 1
            if nparts > 1:
                nc.vector.tensor_add(sums[:, 0:1], sums[:, 0:1], sums[:, 1:2])
            nc.vector.reciprocal(recips[bh][:, i : i + 1], sums[:, 0:1])
            nc.vector.tensor_scalar_mul(
                sa_st[:, 0:W], sa_st[:, 0:W], recips[bh][:, i : i + 1]
            )
            nc.sync.dma_start(sa[bh, i, :, 0:W], sa_st[:, 0:W])

        if i > 0:
            phase3(i - 1)
    phase3(T - 1)


def build_nc():
    if "nc" in _CACHE:
        return _CACHE["nc"]
    from contextlib import ExitStack

    import concourse.bacc as bacc
    import concourse.mybir as mybir
    import concourse.tile as tile

    f32 = mybir.dt.float32
    bf16 = mybir.dt.bfloat16

    nc = bacc.Bacc("TRN2", target_bir_lowering=False, debug=False, num_devices=N_CORES)
    aps = {
        "q": nc.dram_tensor("q", [NBH, L, E], f32, kind="ExternalInput").ap(),
        "k": nc.dram_tensor("k", [NBH, L, E], f32, kind="ExternalInput").ap(),
        "v": nc.dram_tensor("v", [NBH, L, E], f32, kind="ExternalInput").ap(),
        "maskneg": nc.dram_tensor("maskneg", [P, P], f32, kind="ExternalInput").ap(),
        "masknegT": nc.dram_tensor("masknegT", [P, P], f32, kind="ExternalInput").ap(),
        "ident": nc.dram_tensor("ident", [P, P], bf16, kind="ExternalInput").ap(),
        "sa": nc.dram_tensor("sa", [NBH, L, L], f32, kind="ExternalOutput").ap(),
        "o": nc.dram_tensor("o", [NBH, L, E], f32, kind="ExternalOutput").ap(),
    }
    with tile.TileContext(nc) as tc, ExitStack() as ctx:
        _emit(nc, tc, ctx, aps, mybir)
    nc.compile()
    _CACHE["nc"] = nc
    return nc


def _host_consts():
    idx = np.arange(P)
    maskneg = np.where(idx[None, :] <= idx[:, None], 0.0, NEG).astype(np.float32)
    masknegT = maskneg.T.copy()
    ident = np.eye(P, dtype=ml_dtypes.bfloat16)
    return maskneg, masknegT, ident


def make_in_maps(queries, keys, values):
    queries = np.asarray(queries, dtype=np.float32)
    keys = np.asarray(keys, dtype=np.float32)
    values = np.asarray(values, dtype=np.float32)
    maskneg, masknegT, ident = _host_consts()
    in_maps = []
    for c in range(N_CORES):
        pairs = [2 * c, 2 * c + 1]
        qs = np.stack([queries[m // H, :, m % H, :] for m in pairs])
        ks = np.stack([keys[m // H, :, m % H, :] for m in pairs])
        vs = np.stack([values[m // H, :, m % H, :] for m in pairs])
        in_maps.append(
            {
                "q": np.ascontiguousarray(qs),
                "k": np.ascontiguousarray(ks),
                "v": np.ascontiguousarray(vs),
                "maskneg": maskneg,
                "masknegT": masknegT,
                "ident": ident,
            }
        )
    return in_maps


def run(queries, keys, values, trace=False):
    _install_ntff_hook()
    from concourse.bass_utils import run_bass_kernel_spmd

    nc = build_nc()
    in_maps = make_in_maps(queries, keys, values)
    res = run_bass_kernel_spmd(
        nc, in_maps, core_ids=list(range(N_CORES)), trace=trace
    )
    V = np.empty((B, L, H, E), dtype=np.float32)
    SA = np.empty((B, H, L, L), dtype=np.float32)
    for c in range(N_CORES):
        out = res.results[c]
        for idx, m in enumerate([2 * c, 2 * c + 1]):
            b, h = m // H, m % H
            SA[b, h] = out["sa"][idx]
            V[b, :, h, :] = out["o"][idx]
    return (V, SA), res


def kernel(queries, keys, values):
    (V, SA), _ = run(queries, keys, values, trace=False)
    return (V, SA)


# revision 45
# speedup vs baseline: 1.1706x; 1.1706x over previous
"""Causal attention (B=2, L=2048, H=8, E=64) returning (V, SA) on 8 trn2 NeuronCores.

Sharding: the 16 (b,h) pairs are split 2-per-core (batch*head data parallel);
each core holds full L so the causal softmax needs no communication. Only the
lower causal triangle is ever computed or written (the strictly-upper triangle
of SA is never touched: PJRT donates zero-initialized output buffers, so those
regions read back as exact 0).

Per-core pipeline (matmuls bf16 on PE, f32 PSUM accumulation), fully fused over
steps i = 0..15 (row-tile i = 128 query rows), both heads interleaved:
  phase1: S row-tile = Q^T_i.T @ K^T chunks -> additive -1e9 mask on the
          diagonal block (DVE, pre-exp so row sums come out right) -> one ACT
          exp(0.125*x) per PSUM part with accum_out row-sums -> reciprocal ->
          normalize in place (DVE) -> DMA SA rows out immediately.
  phase2: E^T produced directly by recomputing the scores transposed
          (lhsT/rhs swapped) + exp — cheaper than any on-chip transpose path
          (PE-identity transpose needs a PSUM drain pass; DMA transpose is
          packet-rate-bound). Emitted in 1024-wide parts on the causal
          diagonal schedule (part p of col-tile j at step j+8p, just before
          first use) to keep per-step PE/ACT work flat.
  phase3: O row-tile = sum_j E^T-block @ V_j, lagged one step behind phase2
          so the in-order PE never stalls on fresh exp output; normalize by
          the phase-1 reciprocals, DMA out.
Within each step, phase2 is emitted BEFORE phase1 (and phase3 last): measured
~5us faster — prioritizing E^T production keeps the next step's O-matmul
inputs ahead of the pipeline. Input loads are one DMA per tensor with issue
work split across both HWDGE queues (Q/K on the scalar queue, V on sync):
twelve split issues serialized ~10us on one queue and cost ~2us of wall time.
"""
import sys
import types

sys.path.insert(0, "/opt/trn_rl_repo")

import numpy as np
import ml_dtypes

B, L, H, E = 2, 2048, 8, 64
P = 128          # partitions / l-tile size
T = L // P       # 16 tiles
NBH = 2          # (b,h) pairs per core
N_CORES = 8
SCALE = 1.0 / 8.0
NEG = -1.0e9

_CACHE = {}


def _install_ntff_hook():
    """The image's antenv lacks axon_hooks; shim it so trace=True works."""
    import antenv

    if "antenv.axon_hooks" in sys.modules:
        return
    hooks = types.ModuleType("antenv.axon_hooks")
    hooks._hook = None
    hooks.set_axon_ntff_profile_hook = lambda h: setattr(hooks, "_hook", h)
    hooks.get_axon_ntff_profile_hook = lambda: hooks._hook
    sys.modules["antenv.axon_hooks"] = hooks
    antenv.axon_hooks = hooks
    try:
        from trn_agent_boot.trn_boot import _ntff_profile_via_ctypes

        hook = _ntff_profile_via_ctypes("/opt/axon/libaxon_pjrt.so")
        if hook is not None:
            hooks.set_axon_ntff_profile_hook(hook)
    except Exception:
        pass


def _emit(nc, tc, ctx, aps, mybir):
    import concourse.tile as tile  # noqa: F401

    f32 = mybir.dt.float32
    bf16 = mybir.dt.bfloat16
    Exp = mybir.ActivationFunctionType.Exp

    q = aps["q"].rearrange("n (t p) e -> n p t e", p=P)
    k = aps["k"].rearrange("n (t p) e -> n p t e", p=P)
    v = aps["v"].rearrange("n (t p) e -> n p t e", p=P)
    sa = aps["sa"].rearrange("n (t p) s -> n t p s", p=P)
    o = aps["o"].rearrange("n (t p) e -> n t p e", p=P)

    # packed-causal column offsets inside the E / E^T sbuf tiles
    offE = [P * (i * (i + 1) // 2) for i in range(T + 1)]            # row-tile i: width (i+1)*P
    offT = [P * (T * j - j * (j - 1) // 2) for j in range(T + 1)]    # col-tile j: width (T-j)*P
    CW = offE[T]  # 17408

    cpool = ctx.enter_context(tc.tile_pool(name="consts", bufs=1))
    ldpool = ctx.enter_context(tc.tile_pool(name="ld", bufs=3))
    bfpool = ctx.enter_context(tc.tile_pool(name="bfp", bufs=2))
    qktpool = ctx.enter_context(tc.tile_pool(name="qkt", bufs=2))
    bigpool = ctx.enter_context(tc.tile_pool(name="big", bufs=2))
    outpool = ctx.enter_context(tc.tile_pool(name="outp", bufs=8))
    smallpool = ctx.enter_context(tc.tile_pool(name="small", bufs=4))
    pspool = ctx.enter_context(tc.tile_pool(name="ps", bufs=3, space="PSUM"))
    popool = ctx.enter_context(tc.tile_pool(name="po", bufs=2, space="PSUM"))

    PSW = 1024  # S/ST psum tile width

    mask_sb = cpool.tile([P, P], f32, tag="mask")
    nc.sync.dma_start(mask_sb[:], aps["maskneg"][:])
    maskT_sb = cpool.tile([P, P], f32, tag="maskT")
    nc.sync.dma_start(maskT_sb[:], aps["masknegT"][:])
    ident_sb = cpool.tile([P, P], bf16, tag="ident")
    nc.sync.dma_start(ident_sb[:], aps["ident"][:])

    qt = {}
    kt = {}
    vb = {}
    et = {}
    recips = {}
    for bh in range(NBH):
        # ---- load + cast ----
        qf = ldpool.tile([P, T, E], f32, tag="ld")
        nc.scalar.dma_start(qf[:], q[bh])
        kf = ldpool.tile([P, T, E], f32, tag="ld")
        nc.scalar.dma_start(kf[:], k[bh])
        vf = ldpool.tile([P, T, E], f32, tag="ld")
        nc.sync.dma_start(vf[:], v[bh])

        qb = bfpool.tile([P, T, E], bf16, tag="qb")
        nc.vector.tensor_copy(qb[:], qf[:])
        kb = bfpool.tile([P, T, E], bf16, tag="kb")
        nc.vector.tensor_copy(kb[:], kf[:])
        vb[bh] = bfpool.tile([P, T, E], bf16, tag="vb", name=f"vb{bh}")
        nc.vector.tensor_copy(vb[bh][:], vf[:])

        # ---- Q^T / K^T via matmul-by-identity (one batched PSUM->SBUF cast) ----
        qt[bh] = qktpool.tile([E, L], bf16, tag="qt", name=f"qt{bh}")
        kt[bh] = qktpool.tile([E, L], bf16, tag="kt", name=f"kt{bh}")
        for src, dst in ((qb, qt[bh]), (kb, kt[bh])):
            pt = pspool.tile([E, PSW], f32, tag="ps")
            for t in range(8):
                nc.tensor.matmul(
                    pt[:, t * P : (t + 1) * P], src[:, t, :], ident_sb[:],
                    start=True, stop=True,
                )
            nc.vector.tensor_copy(dst[:, 0 : 8 * P], pt[:, 0 : 8 * P])
            pt2 = pspool.tile([E, PSW], f32, tag="ps")
            for t in range(8, T):
                tt = t - 8
                nc.tensor.matmul(
                    pt2[:, tt * P : (tt + 1) * P],
                    src[:, t, :], ident_sb[:], start=True, stop=True,
                )
            nc.vector.tensor_copy(dst[:, 8 * P : L], pt2[:, 0 : L - 8 * P])

        et[bh] = bigpool.tile([P, CW], bf16, tag="et", name=f"et{bh}")
        recips[bh] = smallpool.tile([P, T], f32, tag="recips", name=f"recips{bh}")

    # ---- fused main loop. Step i emits:
    #  * phase1(i): S row-tile i -> mask -> exp(+rowsum accum) -> normalize -> DMA SA
    #  * phase2 parts on the diagonal: 1024-wide row-parts (j, p) with j+8p == i,
    #    exactly when first needed, so per-step ACT/PE work stays flat instead of
    #    front-loading the wide S^T column-tiles
    #  * phase3(i-1): O row-tile lagged one step so its E^T inputs are a step old
    #    and the in-order PE never stalls waiting on fresh exp output
    def phase3(ip):
        for bh in range(NBH):
            po = popool.tile([P, E], f32, tag="po")
            for j3 in range(ip + 1):
                nc.tensor.matmul(
                    po[:],
                    et[bh][:, offT[j3] + (ip - j3) * P : offT[j3] + (ip - j3 + 1) * P],
                    vb[bh][:, j3, :],
                    start=(j3 == 0),
                    stop=(j3 == ip),
                )
            o_st = outpool.tile([P, E], f32, tag="o")
            nc.vector.tensor_scalar_mul(
                o_st[:], po[:], recips[bh][:, ip : ip + 1]
            )
            nc.sync.dma_start(o[bh, ip], o_st[:])

    for i in range(T):
        W = (i + 1) * P
        for bh in range(NBH):
            # phase 2 first
            # phase 2 parts: (j, p) with j + 8p == i
            for (j, pp) in ([(i, 0)] + ([(i - 8, 1)] if i >= 8 else [])):
                Wj = (T - j) * P
                p0 = pp * PSW
                if p0 >= Wj:
                    continue
                pw = min(PSW, Wj - p0)
                base = j * P
                ps = pspool.tile([P, PSW], f32, tag="ps")
                for c0 in range(0, pw, 512):
                    w = min(512, pw - c0)
                    nc.tensor.matmul(
                        ps[:, c0 : c0 + w],
                        kt[bh][:, base : base + P],
                        qt[bh][:, base + p0 + c0 : base + p0 + c0 + w],
                        start=True,
                        stop=True,
                    )
                if pp == 0:  # diag block is the first 128 cols of part 0
                    nc.vector.tensor_add(ps[:, 0:P], ps[:, 0:P], maskT_sb[:])
                nc.scalar.activation(
                    et[bh][:, offT[j] + p0 : offT[j] + p0 + pw],
                    ps[:, 0:pw], Exp, scale=SCALE,
                )

            # phase 1
            sums = smallpool.tile([P, 2], f32, tag="sums")
            sa_st = outpool.tile([P, 2048], f32, tag="sa")
            nparts = 0
            for p0 in range(0, W, PSW):
                pw = min(PSW, W - p0)
                ps = pspool.tile([P, PSW], f32, tag="ps")
                for c0 in range(0, pw, 512):
                    w = min(512, pw - c0)
                    nc.tensor.matmul(
                        ps[:, c0 : c0 + w],
                        qt[bh][:, i * P : (i + 1) * P],
                        kt[bh][:, p0 + c0 : p0 + c0 + w],
                        start=True,
                        stop=True,
                    )
                if p0 <= i * P < p0 + pw:  # diag block lives in this psum tile
                    d0 = i * P - p0
                    nc.vector.tensor_add(
                        ps[:, d0 : d0 + P], ps[:, d0 : d0 + P], mask_sb[:]
                    )
                nc.scalar.activation(
                    sa_st[:, p0 : p0 + pw], ps[:, 0:pw], Exp, scale=SCALE,
                    accum_out=sums[:, nparts : nparts + 1],
                )
                nparts += 1
            if nparts > 1:
                nc.vector.tensor_add(sums[:, 0:1], sums[:, 0:1], sums[:, 1:2])
            nc.vector.reciprocal(recips[bh][:, i : i + 1], sums[:, 0:1])
            nc.vector.tensor_scalar_mul(
                sa_st[:, 0:W], sa_st[:, 0:W], recips[bh][:, i : i + 1]
            )
            nc.sync.dma_start(sa[bh, i, :, 0:W], sa_st[:, 0:W])

        if i > 0:
            phase3(i - 1)
    phase3(T - 1)


def build_nc():
    if "nc" in _CACHE:
        return _CACHE["nc"]
    from contextlib import ExitStack

    import concourse.bacc as bacc
    import concourse.mybir as mybir
    import concourse.tile as tile

    f32 = mybir.dt.float32
    bf16 = mybir.dt.bfloat16

    nc = bacc.Bacc("TRN2", target_bir_lowering=False, debug=False, num_devices=N_CORES)
    aps = {
        "q": nc.dram_tensor("q", [NBH, L, E], f32, kind="ExternalInput").ap(),
        "k": nc.dram_tensor("k", [NBH, L, E], f32, kind="ExternalInput").ap(),
        "v": nc.dram_tensor("v", [NBH, L, E], f32, kind="ExternalInput").ap(),
        "maskneg": nc.dram_tensor("maskneg", [P, P], f32, kind="ExternalInput").ap(),
        "masknegT": nc.dram_tensor("masknegT", [P, P], f32, kind="ExternalInput").ap(),
        "ident": nc.dram_tensor("ident", [P, P], bf16, kind="ExternalInput").ap(),
        "sa": nc.dram_tensor("sa", [NBH, L, L], f32, kind="ExternalOutput").ap(),
        "o": nc.dram_tensor("o", [NBH, L, E], f32, kind="ExternalOutput").ap(),
    }
    with tile.TileContext(nc) as tc, ExitStack() as ctx:
        _emit(nc, tc, ctx, aps, mybir)
    nc.compile()
    _CACHE["nc"] = nc
    return nc


def _host_consts():
    idx = np.arange(P)
    maskneg = np.where(idx[None, :] <= idx[:, None], 0.0, NEG).astype(np.float32)
    masknegT = maskneg.T.copy()
    ident = np.eye(P, dtype=ml_dtypes.bfloat16)
    return maskneg, masknegT, ident


def make_in_maps(queries, keys, values):
    queries = np.asarray(queries, dtype=np.float32)
    keys = np.asarray(keys, dtype=np.float32)
    values = np.asarray(values, dtype=np.float32)
    maskneg, masknegT, ident = _host_consts()
    in_maps = []
    for c in range(N_CORES):
        pairs = [2 * c, 2 * c + 1]
        qs = np.stack([queries[m // H, :, m % H, :] for m in pairs])
        ks = np.stack([keys[m // H, :, m % H, :] for m in pairs])
        vs = np.stack([values[m // H, :, m % H, :] for m in pairs])
        in_maps.append(
            {
                "q": np.ascontiguousarray(qs),
                "k": np.ascontiguousarray(ks),
                "v": np.ascontiguousarray(vs),
                "maskneg": maskneg,
                "masknegT": masknegT,
                "ident": ident,
            }
        )
    return in_maps


def run(queries, keys, values, trace=False):
    _install_ntff_hook()
    from concourse.bass_utils import run_bass_kernel_spmd

    nc = build_nc()
    in_maps = make_in_maps(queries, keys, values)
    res = run_bass_kernel_spmd(
        nc, in_maps, core_ids=list(range(N_CORES)), trace=trace
    )
    V = np.empty((B, L, H, E), dtype=np.float32)
    SA = np.empty((B, H, L, L), dtype=np.float32)
    for c in range(N_CORES):
        out = res.results[c]
        for idx, m in enumerate([2 * c, 2 * c + 1]):
            b, h = m // H, m % H
            SA[b, h] = out["sa"][idx]
            V[b, :, h, :] = out["o"][idx]
    return (V, SA), res


def kernel(queries, keys, values):
    (V, SA), _ = run(queries, keys, values, trace=False)
    return (V, SA)
